# revision 1
# baseline (speedup 1.0000x reference)
"""GroupGAT kernel for Trainium2 (Bass/Tile), 8-core data-parallel.

Math restructure (attention weights commute with @W):
    e[b,n] = lrelu(h_self.(W a1) + h[b,n].(W a2))    <- dots in h-space
    out = elu((h_self + hw_ally) @ W_ally + hw_opp @ W_opp),
    hw_x[b,:] = sum_n w_x[b,n] h[b,n,:]              <- weighted sums in h-space

Measured-fact-driven engine mapping (v2.9):
  - h loaded as ONE flat 2D DMA per tile; hT via full-tile xbar transpose
    (offset-0 APs), ring alternating SP/ACT per tile.
  - dots: 41 PE matmuls lhsT=hT[:,n,:] x rhs=vcat[d,4] -> e_ps[b,n,4]
    (measured 26ns cadence).
  - weighted sums: the diag(w01) matrix is nonzero only in its four 32x32
    diagonal blocks. diagblk [P, 44, 32] holds those blocks (c innermost,
    contiguous for LDWEIGHTS); built by ONE GPSIMD TT multiply against a
    constant block-mask. Per node, 4 concurrent tile-positioned sub-matmuls
    (lhsT=diagblk block, rhs=h_n partition-slice, both contiguous) accumulate
    hw[b, d] in PSUM.
  - finals: hw -> DVE copy bf16 -> small xbar transpose -> lhsT=xT x W.
  - 2-deep software pipeline so PE never waits on the softmax/diag chain.
"""

import numpy as np
import ml_dtypes

import concourse.bass as bass
import concourse.bacc as bacc
import concourse.mybir as mybir
from concourse import tile
from concourse.bass_utils import run_bass_kernel_spmd

N_CORES = 8
B = 16384
NN = 41
NA = 20
NO = 20
D = 128
B_SHARD = B // N_CORES
P = 128
NEG_INF = -1e9
NJ = 44  # diag slots padded even: 0..20 ally, 21..41 opp, 42..43 zero

F32 = mybir.dt.float32
BF16 = mybir.dt.bfloat16
AL = mybir.AluOpType
AF = mybir.ActivationFunctionType
BF16_NP = ml_dtypes.bfloat16

DVE_DIAG = 28  # diag cols 0..27 on DVE; 28..43 on GPSIMD


def _h_node_of_slot(j):
    if j <= 20:
        return j
    if j == 21:
        return 0
    return j - 1  # 22..41 -> h nodes 21..40


def build_nc(b_shard=B_SHARD):
    n_tiles = b_shard // P
    nc = bacc.Bacc("TRN2", target_bir_lowering=False, debug=False)

    h_d = nc.dram_tensor("h", [b_shard, NN * D], BF16, kind="ExternalInput").ap()
    mneg_d = nc.dram_tensor("mneg", [b_shard, 42], F32, kind="ExternalInput").ap()
    vcat_d = nc.dram_tensor("vcat", [D, 4], BF16, kind="ExternalInput").ap()
    wcat_d = nc.dram_tensor("wcat", [D, 2 * D], BF16, kind="ExternalInput").ap()
    maskf_d = nc.dram_tensor("maskf", [P, NJ * D], BF16, kind="ExternalInput").ap()
    ones_d = nc.dram_tensor("ones", [P, 42], F32, kind="ExternalInput").ap()
    out_d = nc.dram_tensor("out", [b_shard, D], F32, kind="ExternalOutput").ap()

    with tile.TileContext(nc) as tc:
        with (
            tc.tile_pool(name="const", bufs=1) as cpool,
            tc.tile_pool(name="hin", bufs=4) as hpool,
            tc.tile_pool(name="ht", bufs=3) as htpool,
            tc.tile_pool(name="diag", bufs=3) as dpool,
            tc.tile_pool(name="small", bufs=4) as spool,
            tc.tile_pool(name="work", bufs=3) as wpool,
            tc.tile_pool(name="psum", bufs=2, space=bass.MemorySpace.PSUM) as ppool,
            tc.tile_pool(name="psum1", bufs=1, space=bass.MemorySpace.PSUM) as ppool1,
        ):
            vcat = cpool.tile([D, 4], BF16)
            wcat = cpool.tile([D, 2 * D], BF16)
            maskf = cpool.tile([P, NJ, D], BF16)
            ones42 = cpool.tile([P, 42], F32)
            nc.sync.dma_start(vcat[:], vcat_d[:])
            nc.sync.dma_start(wcat[:], wcat_d[:])
            nc.sync.dma_start(maskf[:], maskf_d[:])
            nc.sync.dma_start(ones42[:], ones_d[:])

            def phase_a(it):
                b0 = it * P
                h_t = hpool.tile([P, NN * D], BF16)
                mneg_t = spool.tile([P, 42], F32, tag="mneg")
                nc.sync.dma_start(h_t[:], h_d[b0 : b0 + P])
                nc.sync.dma_start(mneg_t[:], mneg_d[b0 : b0 + P])

                # hT[d, n, b] = h[b, n, d]; full-tile xbar, alternate rings
                hT = htpool.tile([P, NN, D], BF16)
                nc.scalar.dma_start_transpose(hT[:], h_t[:])

                # --- dots on PE: e_ps[b, n, g] = sum_d hT[d,n,b] * vcat[d,g]
                e_ps = ppool1.tile([P, NN, 4], F32, tag="eps")
                for n in range(NN):
                    nc.tensor.matmul(
                        e_ps[:, n, :], hT[:, n, :], vcat[:], start=True, stop=True
                    )

                # --- e assembly (cols: 0..20 ally, 21..41 opp) ---
                e_pre = spool.tile([P, 42], F32, tag="epre")
                s1a = e_ps[:, 0:1, 0]
                s1o = e_ps[:, 0:1, 2]
                nc.vector.scalar_tensor_tensor(
                    e_pre[:, 0:21], e_ps[:, 0:21, 1], s1a,
                    mneg_t[:, 0:21], AL.add, AL.add,
                )
                nc.vector.tensor_scalar_add(e_pre[:, 21:22], e_ps[:, 0:1, 3], s1o)
                nc.vector.scalar_tensor_tensor(
                    e_pre[:, 22:42], e_ps[:, 21:NN, 3], s1o,
                    mneg_t[:, 22:42], AL.add, AL.add,
                )
                nc.vector.scalar_tensor_tensor(
                    e_pre[:], e_pre[:], 0.2, e_pre[:], AL.mult, AL.max
                )

                # --- softmax weights (w01 bf16 [P, 44], pad cols zeroed) ---
                expe = spool.tile([P, 42], F32, tag="expe")
                den = spool.tile([P, 2], F32, tag="den")
                rec = spool.tile([P, 2], F32, tag="rec")
                nc.scalar.activation(
                    expe[:, 0:21], e_pre[:, 0:21], AF.Exp, accum_out=den[:, 0:1]
                )
                nc.scalar.activation(
                    expe[:, 21:42], e_pre[:, 21:42], AF.Exp, accum_out=den[:, 1:2]
                )
                nc.vector.reciprocal(rec[:], den[:])
                w01 = spool.tile([P, NJ], BF16, tag="w01")
                nc.vector.scalar_tensor_tensor(
                    w01[:, 0:21], expe[:, 0:21], rec[:, 0:1],
                    ones42[:, 0:21], AL.mult, AL.mult,
                )
                nc.vector.scalar_tensor_tensor(
                    w01[:, 21:42], expe[:, 21:42], rec[:, 1:2],
                    ones42[:, 21:42], AL.mult, AL.mult,
                )
                nc.vector.tensor_scalar_add(w01[:, 0:1], w01[:, 0:1], 1.0)
                nc.vector.memset(w01[:, 42:44], 0.0)

                # --- diag materialization: diag[p, j, d] = w01[p, j]*(d == p)
                # j-outer layout so the wsum matmul moving operand is contiguous
                diag = dpool.tile([P, NJ, D], BF16)
                nc.vector.tensor_mul(
                    diag[:, 0:DVE_DIAG, :],
                    maskf[:, 0:DVE_DIAG, :],
                    w01[:, 0:DVE_DIAG, None].broadcast_to([P, DVE_DIAG, D]),
                )
                nc.gpsimd.affine_select(
                    diag[:, DVE_DIAG:NJ, :],
                    w01[:, DVE_DIAG:NJ, None].broadcast_to([P, NJ - DVE_DIAG, D]),
                    pattern=[[0, NJ - DVE_DIAG], [1, D]],
                    compare_op=AL.is_equal,
                    fill=0.0,
                    base=0,
                    channel_multiplier=-1,
                )
                return h_t, diag

            def phase_b(it, h_t, diag):
                b0 = it * P

                def hnode(n):
                    return h_t[:, n * D : (n + 1) * D]

                # --- weighted sums on PE: hwT[d, b] += h_n[b,d]*w01[b,j]
                hwps = ppool.tile([P, 2, D], F32, tag="hw")
                hwT_a = hwps[:, 0, :]
                hwT_o = hwps[:, 1, :]
                for grp, hwT in ((0, hwT_a), (1, hwT_o)):
                    for k in range(21):
                        j = grp * 21 + k
                        nc.tensor.matmul(
                            hwT, hnode(_h_node_of_slot(j)), diag[:, j, :],
                            start=(k == 0), stop=(k == 20),
                        )

                xT_a = wpool.tile([P, D], BF16, tag="xta")
                xT_o = wpool.tile([P, D], BF16, tag="xto")
                nc.scalar.copy(xT_a[:], hwT_a)
                nc.scalar.copy(xT_o[:], hwT_o)

                # --- out = elu(xT_a.T @ W_ally + xT_o.T @ W_opp)
                out_ps = ppool.tile([P, D], F32, tag="ops")
                nc.tensor.matmul(out_ps[:], xT_a[:], wcat[:, 0:D], start=True, stop=False)
                nc.tensor.matmul(out_ps[:], xT_o[:], wcat[:, D : 2 * D], start=False, stop=True)

                # elu(x) = max(x, exp(min(x,0)) - 1)
                t1 = wpool.tile([P, D], F32, tag="t1")
                out_t = wpool.tile([P, D], F32, tag="outt")
                nc.vector.tensor_scalar_min(t1[:], out_ps[:], 0.0)
                nc.scalar.activation(t1[:], t1[:], AF.Exp)
                nc.vector.scalar_tensor_tensor(
                    out_t[:], t1[:], -1.0, out_ps[:], AL.add, AL.max
                )
                nc.sync.dma_start(out_d[b0 : b0 + P], out_t[:])

            states = {}
            for it in range(n_tiles):
                states[it] = phase_a(it)
                if it >= 2:
                    phase_b(it - 2, *states.pop(it - 2))
            for it in (n_tiles - 2, n_tiles - 1):
                phase_b(it, *states.pop(it))

    nc.compile()
    return nc


_NC_CACHE = {}


def _get_nc(b_shard):
    if b_shard not in _NC_CACHE:
        _NC_CACHE[b_shard] = build_nc(b_shard)
    return _NC_CACHE[b_shard]


def _host_precompute(W_ally, W_opp, a_ally, a_opp, mask):
    v1a = W_ally @ a_ally[:D, 0]
    v2a = W_ally @ a_ally[D:, 0]
    v1o = W_opp @ a_opp[:D, 0]
    v2o = W_opp @ a_opp[D:, 0]
    vcat = np.ascontiguousarray(np.stack([v1a, v2a, v1o, v2o], axis=1).astype(BF16_NP))
    wcat = np.ascontiguousarray(np.concatenate([W_ally, W_opp], axis=1).astype(BF16_NP))
    eye = (np.arange(P)[:, None] == np.arange(D)[None, :]).astype(BF16_NP)
    maskf = np.ascontiguousarray(
        np.repeat(eye[:, None, :], NJ, axis=1).reshape(P, NJ * D)
    )
    ones = np.ones((P, 42), np.float32)
    b = mask.shape[0]
    mneg = np.zeros((b, 42), np.float32)
    mneg[:, 1:21] = np.where(mask[:, 1 : 1 + NA], NEG_INF, 0.0)
    mneg[:, 22:42] = np.where(mask[:, 1 + NA :], NEG_INF, 0.0)
    return vcat, wcat, maskf, ones, mneg


def kernel(h, W_ally, W_opp, a_ally, a_opp, mask, num_ally, num_opp):
    assert int(num_ally) == NA and int(num_opp) == NO
    h = np.asarray(h, dtype=np.float32)
    mask = np.asarray(mask)
    W_ally = np.asarray(W_ally, dtype=np.float32)
    W_opp = np.asarray(W_opp, dtype=np.float32)
    a_ally = np.asarray(a_ally, dtype=np.float32)
    a_opp = np.asarray(a_opp, dtype=np.float32)

    vcat, wcat, maskf, ones, mneg = _host_precompute(W_ally, W_opp, a_ally, a_opp, mask)
    bfull = h.shape[0]
    h_bf = np.ascontiguousarray(h.reshape(bfull, NN * D).astype(BF16_NP))

    nc = _get_nc(B_SHARD)
    in_maps = []
    for c in range(N_CORES):
        s = slice(c * B_SHARD, (c + 1) * B_SHARD)
        in_maps.append(
            {
                "h": h_bf[s],
                "mneg": np.ascontiguousarray(mneg[s]),
                "vcat": vcat,
                "wcat": wcat,
                "maskf": maskf,
                "ones": ones,
            }
        )
    res = run_bass_kernel_spmd(nc, in_maps, core_ids=list(range(N_CORES)))
    global LAST_RESULTS
    LAST_RESULTS = res
    return np.concatenate([res.results[c]["out"] for c in range(N_CORES)], axis=0)


LAST_RESULTS = None



# revision 3
# speedup vs baseline: 1.1753x; 1.1753x over previous
"""GroupGAT kernel for Trainium2 (Bass/Tile), 8-core data-parallel.

Math restructure (attention weights commute with @W):
    e[b,n] = lrelu(h_self.(W a1) + h[b,n].(W a2))    <- dots in h-space
    out = elu((h_self + hw_ally) @ W_ally + hw_opp @ W_opp),
    hw_x[b,:] = sum_n w_x[b,n] h[b,n,:]              <- weighted sums in h-space

v4 "dual-load" design (per 128-row tile):
  - NO on-chip transpose. The host supplies BOTH layouts of h from HBM:
    h_t [b, n*d] (b-part, wsum stationaries) and hT [d, n, b] (d-part,
    dot stationaries). 2x HBM reads beat the xbar transpose: the xbar
    moves 256B/descriptor at ~13.6GB/s/queue vs ~21GB/s/queue for HBM
    loads, and the transpose also blocked the ACT engine ~6.2us/tile.
  - dots: 41 PE matmuls lhsT=hT[:,n,:] x rhs=vcat[d,4] -> e_ps[b,n,4].
  - diag[p,j,d] = w01[p,j]*(d==p) materialized split across three
    engines (DVE 20 slots / ACT 6 / GPSIMD 16) so no engine exceeds
    the DMA cadence.
  - wsums: 42 PE matmuls lhsT=h_j (b-part) x rhs=diag_j -> hwT[d,b]
    accumulated in PSUM (diag trick scales + transposes in one pass).
  - finals: hwT -> ACT copy bf16 -> lhsT=xT x wcat; ELU; store.
  - 3-stage software pipeline (load / dots+e+diag / wsum+fin) with
    2-tile DMA prefetch; per-engine streams ordered oldest-deps-first.
"""

import numpy as np
import ml_dtypes

import concourse.bass as bass
import concourse.bacc as bacc
import concourse.mybir as mybir
from concourse import tile
from concourse.bass_utils import run_bass_kernel_spmd

N_CORES = 8
B = 16384
NN = 41
NA = 20
NO = 20
D = 128
B_SHARD = B // N_CORES
P = 128
NEG_INF = -1e9
NJ = 42  # diag slots: 0..20 ally (h nodes 0..20), 21..41 opp (h nodes 0,21..40)

F32 = mybir.dt.float32
BF16 = mybir.dt.bfloat16
AL = mybir.AluOpType
AF = mybir.ActivationFunctionType
BF16_NP = ml_dtypes.bfloat16

# diag slot split across engines
DVE_NSLOT = 20   # slots 0..19 on DVE
ACT_NSLOT = 6    # slots 20..25 on ACT (per-slot scale-copy)
GPS_LO = DVE_NSLOT + ACT_NSLOT  # 26: slots 26..41 on GPSIMD


def _h_node_of_slot(j):
    if j <= 20:
        return j
    if j == 21:
        return 0
    return j - 1  # 22..41 -> h nodes 21..40


def build_nc(b_shard=B_SHARD):
    n_tiles = b_shard // P
    nc = bacc.Bacc("TRN2", target_bir_lowering=False, debug=False)

    h_d = nc.dram_tensor("h", [b_shard, NN * D], BF16, kind="ExternalInput").ap()
    hT_d = nc.dram_tensor("hT", [b_shard, NN * D], BF16, kind="ExternalInput").ap()
    mneg_d = nc.dram_tensor("mneg", [b_shard, NJ], F32, kind="ExternalInput").ap()
    vcat_d = nc.dram_tensor("vcat", [D, 4], BF16, kind="ExternalInput").ap()
    wcat_d = nc.dram_tensor("wcat", [D, 2 * D], BF16, kind="ExternalInput").ap()
    maskf_d = nc.dram_tensor("maskf", [P, NJ * D], BF16, kind="ExternalInput").ap()
    ones_d = nc.dram_tensor("ones", [P, NJ], F32, kind="ExternalInput").ap()
    out_d = nc.dram_tensor("out", [b_shard, D], F32, kind="ExternalOutput").ap()

    with tile.TileContext(nc) as tc:
        with (
            tc.tile_pool(name="const", bufs=1) as cpool,
            tc.tile_pool(name="hin", bufs=5) as hpool,
            tc.tile_pool(name="htin", bufs=4) as htpool,
            tc.tile_pool(name="diag", bufs=3) as dpool,
            tc.tile_pool(name="small", bufs=4) as spool,
            tc.tile_pool(name="work", bufs=3) as wpool,
            tc.tile_pool(name="psum_e", bufs=2, space=bass.MemorySpace.PSUM) as ppool_e,
            tc.tile_pool(name="psum_hw", bufs=2, space=bass.MemorySpace.PSUM) as ppool_hw,
            tc.tile_pool(name="psum_o", bufs=2, space=bass.MemorySpace.PSUM) as ppool_o,
        ):
            vcat = cpool.tile([D, 4], BF16)
            wcat = cpool.tile([D, 2 * D], BF16)
            maskf = cpool.tile([P, NJ, D], BF16)
            ones42 = cpool.tile([P, NJ], F32)
            nc.sync.dma_start(vcat[:], vcat_d[:])
            nc.sync.dma_start(wcat[:], wcat_d[:])
            nc.sync.dma_start(maskf[:], maskf_d[:])
            nc.sync.dma_start(ones42[:], ones_d[:])

            state = {}

            def phase_load(it):
                b0 = it * P
                h_t = hpool.tile([P, NN * D], BF16)
                hT = htpool.tile([P, NN, D], BF16)  # hT[d, n, b]
                mneg_t = spool.tile([P, NJ], F32, tag="mneg")
                nc.sync.dma_start(h_t[:], h_d[b0 : b0 + P])
                nc.sync.dma_start(hT[:], hT_d[b0 : b0 + P])
                nc.sync.dma_start(mneg_t[:], mneg_d[b0 : b0 + P])
                state[it] = [h_t, hT, mneg_t]

            def phase_mid(it):
                h_t, hT, mneg_t = state[it]

                # --- dots on PE: e_ps[b, n, g] = sum_d hT[d,n,b] * vcat[d,g]
                e_ps = ppool_e.tile([P, NN, 4], F32, tag="eps")
                for n in range(NN):
                    nc.tensor.matmul(
                        e_ps[:, n, :], hT[:, n, :], vcat[:], start=True, stop=True
                    )

                # --- e assembly (cols: 0..20 ally, 21..41 opp) ---
                e_pre = spool.tile([P, NJ], F32, tag="epre")
                s1a = e_ps[:, 0:1, 0]
                s1o = e_ps[:, 0:1, 2]
                nc.vector.scalar_tensor_tensor(
                    e_pre[:, 0:21], e_ps[:, 0:21, 1], s1a,
                    mneg_t[:, 0:21], AL.add, AL.add,
                )
                nc.vector.tensor_scalar_add(e_pre[:, 21:22], e_ps[:, 0:1, 3], s1o)
                nc.vector.scalar_tensor_tensor(
                    e_pre[:, 22:42], e_ps[:, 21:NN, 3], s1o,
                    mneg_t[:, 22:42], AL.add, AL.add,
                )
                nc.vector.scalar_tensor_tensor(
                    e_pre[:], e_pre[:], 0.2, e_pre[:], AL.mult, AL.max
                )

                # --- softmax weights (w01 bf16 [P, 42]) ---
                expe = spool.tile([P, NJ], F32, tag="expe")
                den = spool.tile([P, 2], F32, tag="den")
                rec = spool.tile([P, 2], F32, tag="rec")
                nc.scalar.activation(
                    expe[:, 0:21], e_pre[:, 0:21], AF.Exp, accum_out=den[:, 0:1]
                )
                nc.scalar.activation(
                    expe[:, 21:42], e_pre[:, 21:42], AF.Exp, accum_out=den[:, 1:2]
                )
                nc.vector.reciprocal(rec[:], den[:])
                w01 = spool.tile([P, NJ], F32, tag="w01")
                nc.vector.scalar_tensor_tensor(
                    w01[:, 0:21], expe[:, 0:21], rec[:, 0:1],
                    ones42[:, 0:21], AL.mult, AL.mult,
                )
                nc.vector.scalar_tensor_tensor(
                    w01[:, 21:42], expe[:, 21:42], rec[:, 1:2],
                    ones42[:, 21:42], AL.mult, AL.mult,
                )
                nc.vector.tensor_scalar_add(w01[:, 0:1], w01[:, 0:1], 1.0)

                # --- diag materialization: diag[p, j, d] = w01[p, j]*(d == p)
                # split across DVE / ACT / GPSIMD to balance engine load
                diag = dpool.tile([P, NJ, D], BF16)
                nc.vector.tensor_mul(
                    diag[:, 0:DVE_NSLOT, :],
                    maskf[:, 0:DVE_NSLOT, :],
                    w01[:, 0:DVE_NSLOT, None].broadcast_to([P, DVE_NSLOT, D]),
                )
                for s in range(DVE_NSLOT, GPS_LO):
                    nc.scalar.mul(diag[:, s, :], maskf[:, s, :], w01[:, s : s + 1])
                nc.gpsimd.affine_select(
                    diag[:, GPS_LO:NJ, :],
                    w01[:, GPS_LO:NJ, None].broadcast_to([P, NJ - GPS_LO, D]),
                    pattern=[[0, NJ - GPS_LO], [1, D]],
                    compare_op=AL.is_equal,
                    fill=0.0,
                    base=0,
                    channel_multiplier=-1,
                )
                state[it] = [h_t, diag]

            def phase_out(it):
                h_t, diag = state.pop(it)
                b0 = it * P

                def hnode(n):
                    return h_t[:, n * D : (n + 1) * D]

                # --- weighted sums on PE: hwT[d, b] += h_n[b,d]*w01[b,j]
                hwps = ppool_hw.tile([P, 2, D], F32, tag="hw")
                hwT_a = hwps[:, 0, :]
                hwT_o = hwps[:, 1, :]
                for grp, hwT in ((0, hwT_a), (1, hwT_o)):
                    for k in range(21):
                        j = grp * 21 + k
                        nc.tensor.matmul(
                            hwT, hnode(_h_node_of_slot(j)), diag[:, j, :],
                            start=(k == 0), stop=(k == 20),
                        )

                xT_a = wpool.tile([P, D], BF16, tag="xta")
                xT_o = wpool.tile([P, D], BF16, tag="xto")
                nc.scalar.copy(xT_a[:], hwT_a)
                nc.scalar.copy(xT_o[:], hwT_o)

                # --- out = elu(xT_a.T @ W_ally + xT_o.T @ W_opp)
                out_ps = ppool_o.tile([P, D], F32, tag="ops")
                nc.tensor.matmul(out_ps[:], xT_a[:], wcat[:, 0:D], start=True, stop=False)
                nc.tensor.matmul(out_ps[:], xT_o[:], wcat[:, D : 2 * D], start=False, stop=True)

                # elu(x) = max(x, exp(min(x,0)) - 1)
                t1 = wpool.tile([P, D], F32, tag="t1")
                out_t = wpool.tile([P, D], F32, tag="outt")
                nc.vector.tensor_scalar_min(t1[:], out_ps[:], 0.0)
                nc.scalar.activation(t1[:], t1[:], AF.Exp)
                nc.vector.scalar_tensor_tensor(
                    out_t[:], t1[:], -1.0, out_ps[:], AL.add, AL.max
                )
                nc.sync.dma_start(out_d[b0 : b0 + P], out_t[:])

            # 3-stage pipeline: load(k) / mid(k-2) / out(k-4)
            for k in range(n_tiles + 4):
                if k < n_tiles:
                    phase_load(k)
                if 4 <= k:
                    phase_out(k - 4)
                if 2 <= k < n_tiles + 2:
                    phase_mid(k - 2)

    nc.compile()
    return nc


_NC_CACHE = {}


def _get_nc(b_shard):
    if b_shard not in _NC_CACHE:
        _NC_CACHE[b_shard] = build_nc(b_shard)
    return _NC_CACHE[b_shard]


def _host_precompute(W_ally, W_opp, a_ally, a_opp, mask):
    v1a = W_ally @ a_ally[:D, 0]
    v2a = W_ally @ a_ally[D:, 0]
    v1o = W_opp @ a_opp[:D, 0]
    v2o = W_opp @ a_opp[D:, 0]
    vcat = np.ascontiguousarray(np.stack([v1a, v2a, v1o, v2o], axis=1).astype(BF16_NP))
    wcat = np.ascontiguousarray(np.concatenate([W_ally, W_opp], axis=1).astype(BF16_NP))
    eye = (np.arange(P)[:, None] == np.arange(D)[None, :]).astype(BF16_NP)
    maskf = np.ascontiguousarray(
        np.repeat(eye[:, None, :], NJ, axis=1).reshape(P, NJ * D)
    )
    ones = np.ones((P, NJ), np.float32)
    b = mask.shape[0]
    mneg = np.zeros((b, NJ), np.float32)
    mneg[:, 1:21] = np.where(mask[:, 1 : 1 + NA], NEG_INF, 0.0)
    mneg[:, 22:42] = np.where(mask[:, 1 + NA :], NEG_INF, 0.0)
    return vcat, wcat, maskf, ones, mneg


def kernel(h, W_ally, W_opp, a_ally, a_opp, mask, num_ally, num_opp):
    assert int(num_ally) == NA and int(num_opp) == NO
    h = np.asarray(h, dtype=np.float32)
    mask = np.asarray(mask)
    W_ally = np.asarray(W_ally, dtype=np.float32)
    W_opp = np.asarray(W_opp, dtype=np.float32)
    a_ally = np.asarray(a_ally, dtype=np.float32)
    a_opp = np.asarray(a_opp, dtype=np.float32)

    vcat, wcat, maskf, ones, mneg = _host_precompute(W_ally, W_opp, a_ally, a_opp, mask)
    bfull = h.shape[0]
    h_bf3 = h.astype(BF16_NP)  # [B, NN, D]
    h_bf = np.ascontiguousarray(h_bf3.reshape(bfull, NN * D))
    # hT layout: per tile t of 128 rows, hT[t*128 + d, n*128 + b] = h[t*128 + b, n, d]
    n_tiles_full = bfull // P
    hT_bf = np.ascontiguousarray(
        h_bf3.reshape(n_tiles_full, P, NN, D).transpose(0, 3, 2, 1)
    ).reshape(bfull, NN * D)

    nc = _get_nc(B_SHARD)
    in_maps = []
    for c in range(N_CORES):
        s = slice(c * B_SHARD, (c + 1) * B_SHARD)
        in_maps.append(
            {
                "h": h_bf[s],
                "hT": hT_bf[s],
                "mneg": np.ascontiguousarray(mneg[s]),
                "vcat": vcat,
                "wcat": wcat,
                "maskf": maskf,
                "ones": ones,
            }
        )
    res = run_bass_kernel_spmd(nc, in_maps, core_ids=list(range(N_CORES)))
    global LAST_RESULTS
    LAST_RESULTS = res
    return np.concatenate([res.results[c]["out"] for c in range(N_CORES)], axis=0)


LAST_RESULTS = None
